# revision 17
# baseline (speedup 1.0000x reference)
"""CompressionTransformerLayer on 8 TRN2 NeuronCores (Bass/Tile), v2.1.

Sharding: SEQUENCE-PARALLEL cross-attention (the memory-bound core of the op).
 - Each core streams only 1/8 of the context (4 batches x 1024 tokens, fp8,
   4.2MB) and computes K/V projections, transposed-score softmax partials and
   P@V partial accumulators for ALL 16 heads over its slice. Partials
   (64 V-features + 1 exp-sum row per head) are combined with a single
   ReduceScatter so each core ends up owning 128 query tokens.
 - Self-attention stays tensor-parallel by heads (2 heads/core) with an
   AllGather of head outputs; sa_out/cq projections are replicated (cheap).
 - Tail (co-proj, LN3, FFN) runs token-parallel on each core's own 128
   tokens; host concatenates the 8 x [128, 1024] outputs.
K/V chunk projections are emitted interleaved with the front phases so the
TensorE stays busy during LN chains and the AllGather. LN gains/biases are
folded into the consuming projection weights on the host. ck/cv/ctx are fp8
(validated numerically: rel err ~2e-3 vs 2e-2 budget); everything else bf16
with fp32 PSUM accumulation; softmax accumulators bf16.
"""
import sys
sys.path.insert(0, "/opt/trn_rl_repo")
sys.path.insert(0, "/root/.axon_site")

import contextlib
import os
import numpy as np

import concourse.bass as bass
import concourse.mybir as mybir
import concourse.tile as tile
from concourse import bacc
from concourse.bass import IndirectOffsetOnAxis
from concourse.bass_utils import run_bass_kernel_spmd

f32, bf16 = mybir.dt.float32, mybir.dt.bfloat16
fp8 = mybir.dt.float8e4
i32 = mybir.dt.int32
AF = mybir.ActivationFunctionType
ALU = mybir.AluOpType
BF16NP = mybir.dt.np(bf16)
FP8NP = mybir.dt.np(fp8)

D, H, HD, DFF = 1024, 16, 64, 4096
B, Q, S = 4, 256, 8192
NC = 8
T = B * Q            # 1024 flattened query tokens
HPC = H // NC        # 2 heads per core (self-attention)
FO = D // 128        # 8 feature tiles
SPC = S // NC        # 1024 context tokens per batch per core
SCH = 512            # context chunk (tokens)
NCH = B * SPC // SCH # 8 chunks per core (2 per batch)
EPS = 1e-5

_CACHE = {}


def _build():
    nc = bacc.Bacc("TRN2", target_bir_lowering=False, debug=False,
                   enable_asserts=True, num_devices=NC)

    def din(name, shape, dt=bf16):
        return nc.dram_tensor(name, shape, dt, kind="ExternalInput").ap()

    queries = din("queries", [T, D], f32)
    ctx = din("ctx", [D, B * SPC], fp8)           # per-core slice, feature-major
    wq = din("wq", [D, 128]); wk = din("wk", [D, 128]); wv = din("wv", [D, 128])
    bq = din("bq", [128], f32); bk = din("bk", [128], f32)
    saow = din("saow", [D, D]); saob = din("saob", [D], bf16)
    cqw = din("cqw", [D, D]); ckw = din("ckw", [D, D], fp8); cvw = din("cvw", [D, D], fp8)
    cqb = din("cqb", [128, FO], f32); ckb = din("ckb", [128, FO], f32)
    cow = din("cow", [D, D]); cob = din("cob", [D], bf16)
    w1 = din("w1", [D, DFF]); b1 = din("b1", [128, 32], f32)
    w2 = din("w2", [DFF, D]); b2 = din("b2", [D], bf16)
    own_idx = din("own_idx", [128, 1], i32)

    out = nc.dram_tensor("out", [128, D], f32, kind="ExternalOutput").ap()
    DBG = bool(os.environ.get("BASSDBG"))
    dbg = {}
    if DBG:
        for nm, shp, dt in [("dbg_x", [T, D], f32), ("dbg_q2", [128, FO, T], bf16),
                            ("dbg_acc", [65, H, 256], bf16),
                            ("dbg_rs", [65, H, 128], bf16),
                            ("dbg_x3", [128, D], f32),
                            ("dbg_oself", [128, T], bf16)]:
            dbg[nm] = nc.dram_tensor(nm, shp, dt, kind="ExternalOutput").ap()

    ctx_r = ctx.rearrange("(f fi) t -> fi f t", fi=128)
    ckw_r = ckw.rearrange("(f fi) o -> fi f o", fi=128)
    cvw_r = cvw.rearrange("(f fi) o -> fi f o", fi=128)
    cqw_r = cqw.rearrange("(f fi) o -> fi f o", fi=128)
    saow_r = saow.rearrange("(s fi) n -> fi s n", fi=128)
    cow_r = cow.rearrange("(s fi) n -> fi s n", fi=128)
    w1_r = w1.rearrange("(f fi) n -> fi f n", fi=128)
    w2_r = w2.rearrange("(dt fi) n -> fi dt n", fi=128)

    with tile.TileContext(nc) as tc:
        with contextlib.ExitStack() as ctxs:
            const = ctxs.enter_context(tc.tile_pool(name="const", bufs=1))
            fm = ctxs.enter_context(tc.tile_pool(name="fm", bufs=1))
            q2p = ctxs.enter_context(tc.tile_pool(name="q2p", bufs=1))
            selfp = ctxs.enter_context(tc.tile_pool(name="selfp", bufs=1))
            stream = ctxs.enter_context(tc.tile_pool(name="stream", bufs=3))
            kcp = ctxs.enter_context(tc.tile_pool(name="kcp", bufs=3))
            vap = ctxs.enter_context(tc.tile_pool(name="vap", bufs=2))
            ptp = ctxs.enter_context(tc.tile_pool(name="ptp", bufs=2))
            accp = ctxs.enter_context(tc.tile_pool(name="accp", bufs=1))
            tailp = ctxs.enter_context(tc.tile_pool(name="tailp", bufs=1))
            wstream = ctxs.enter_context(tc.tile_pool(name="wstream", bufs=2))
            sb = ctxs.enter_context(tc.tile_pool(name="sb", bufs=2))
            dram = ctxs.enter_context(tc.tile_pool(name="dram", bufs=1, space="DRAM"))
            psA = ctxs.enter_context(tc.tile_pool(name="psA", bufs=2, space="PSUM"))
            psS = ctxs.enter_context(tc.tile_pool(name="psS", bufs=2, space="PSUM"))
            psV = ctxs.enter_context(tc.tile_pool(name="psV", bufs=1, space="PSUM"))

            def ldconst(ap_, shape, dt, name):
                t = const.tile(shape, dt, tag=name)
                nc.sync.dma_start(t[:], ap_)
                return t

            wq_sb = ldconst(wq.rearrange("(f fi) o -> fi f o", fi=128), [128, FO, 128], bf16, "wq_sb")
            wk_sb = ldconst(wk.rearrange("(f fi) o -> fi f o", fi=128), [128, FO, 128], bf16, "wk_sb")
            wv_sb = ldconst(wv.rearrange("(f fi) o -> fi f o", fi=128), [128, FO, 128], bf16, "wv_sb")
            ck_sb = ldconst(ckw_r, [128, FO, D], fp8, "ck_sb")
            cv_sb = ldconst(cvw_r, [128, FO, D], fp8, "cv_sb")

            bq_sb = ldconst(bq[:, None], [128, 1], f32, "bq_sb")
            bk_sb = ldconst(bk[:, None], [128, 1], f32, "bk_sb")
            cqb_sb = ldconst(cqb[:], [128, FO], f32, "cqb_sb")
            ckb_sb = ldconst(ckb[:], [128, FO], f32, "ckb_sb")
            b1_sb = ldconst(b1[:], [128, 32], f32, "b1_sb")
            idx_sb = ldconst(own_idx[:], [128, 1], i32, "idx_sb")

            def bcast_vec(ap_, n, name):
                full = const.tile([128, n], bf16, tag=name)
                nc.sync.dma_start(full[:], ap_[None, :].to_broadcast((128, n)))
                return full

            saob_bc = bcast_vec(saob, D, "saob_bc")
            cob_bc = bcast_vec(cob, D, "cob_bc")
            b2_bc = bcast_vec(b2, D, "b2_bc")

            dram_x = dram.tile([T, D], f32)
            qn_dram = dram.tile([T, D], bf16)
            xn2_dram = dram.tile([T, D], bf16)
            dram_acc = dram.tile([NC, 65, H, 128], bf16)

            def ln_tile(xt, xn_out, tag_suffix):
                """LayerNorm (normalize only) one [128, 1024] f32 SBUF tile."""
                stats = sb.tile([128, 2, 6], f32, tag="ln_stats", name=f"ln_st_{tag_suffix}")
                nc.vector.bn_stats(stats[:, 0, :], xt[:, 0:512])
                nc.vector.bn_stats(stats[:, 1, :], xt[:, 512:1024])
                mv = sb.tile([128, 2], f32, tag="ln_mv", name=f"ln_mv_{tag_suffix}")
                nc.vector.bn_aggr(mv[:], stats[:])
                eps = sb.tile([128, 1], f32, tag="ln_eps", name=f"ln_eps_{tag_suffix}")
                nc.vector.memset(eps[:], EPS)
                rstd = sb.tile([128, 1], f32, tag="ln_rstd", name=f"ln_rs_{tag_suffix}")
                nc.scalar.activation(rstd[:], mv[:, 1:2], AF.Sqrt, bias=eps[:], scale=1.0)
                nc.vector.reciprocal(rstd[:], rstd[:])
                nc.vector.tensor_scalar(xn_out, xt[:], scalar1=mv[:, 0:1], scalar2=rstd[:],
                                        op0=ALU.subtract, op1=ALU.mult)

            def ln_to_dram(src_dram, out_dram, ntt, tag):
                for tt in range(ntt):
                    xt = sb.tile([128, 1024], f32, tag="ln_in", name=f"ln_in_{tag}{tt}")
                    nc.sync.dma_start(xt[:], src_dram[tt * 128:(tt + 1) * 128, :])
                    xn = sb.tile([128, 1024], bf16, tag="ln_xn", name=f"ln_xn_{tag}{tt}")
                    ln_tile(xt, xn[:], f"{tag}{tt}")
                    nc.sync.dma_start(out_dram[tt * 128:(tt + 1) * 128, :], xn[:])

            def transpose_load(dst, src_dram):
                # per-[128,128]-block transposes so they pipeline with the LN tiles
                for fo in range(FO):
                    for tt in range(FO):
                        nc.sync.dma_start_transpose(
                            dst[:, fo, tt * 128:(tt + 1) * 128],
                            src_dram[tt * 128:(tt + 1) * 128, fo * 128:(fo + 1) * 128])

            # ---- cross-attn K projection for one chunk (emitted early/interleaved)
            ctx_t, kc_t, vaug_t = {}, {}, {}

            def k_chunk(sc):
                base = (sc // 2) * SPC + (sc % 2) * SCH
                ctx_T = stream.tile([128, FO, SCH], fp8, tag="ctx_T", name=f"ctxT_{sc}")
                nc.sync.dma_start(ctx_T[:], ctx_r[:, :, base:base + SCH])
                kc = kcp.tile([128, FO, SCH], bf16, tag="kc", name=f"kc{sc}")
                for p in range(FO):
                    psk = psA.tile([128, 512], f32, tag="ps512", name=f"ps_k{sc}_{p}")
                    for f in range(FO):
                        nc.tensor.matmul(psk[:], ck_sb[:, f, p * 128:(p + 1) * 128],
                                         ctx_T[:, f, :],
                                         start=(f == 0), stop=(f == FO - 1))
                    nc.scalar.activation(kc[:, p, :], psk[:], AF.Identity,
                                         bias=ckb_sb[:, p:p + 1])
                ctx_t[sc], kc_t[sc] = ctx_T, kc

            # ---- V projection (deferred until q2 is ready, right before scores)
            def v_chunk(sc):
                ctx_T = ctx_t.pop(sc)
                vaug = vap.tile([128, 4, H, 65], bf16, tag="vaug", name=f"vaug{sc}")
                nc.vector.memset(vaug[:, :, :, 64:65], 1.0)
                for half in range(2):
                    for kt in range(4):
                        psv = psA.tile([128, 8, 64], f32, tag="ps512",
                                       name=f"ps_v{sc}_{kt}_{half}")
                        for f in range(FO):
                            nc.tensor.matmul(psv[:], ctx_T[:, f, kt * 128:(kt + 1) * 128],
                                             cv_sb[:, f, half * 512:(half + 1) * 512],
                                             start=(f == 0), stop=(f == FO - 1))
                        nc.vector.tensor_copy(vaug[:, kt, half * 8:(half + 1) * 8, 0:64],
                                              psv[:])
                vaug_t[sc] = vaug

            # ---- scores + PV for one chunk, accumulate into acc
            def spv_chunk(sc, acc):
                b, j = sc // 2, sc % 2
                kc, vaug = kc_t.pop(sc), vaug_t.pop(sc)
                for g in range(8):          # head pair: even rows 0:64, odd 64:128
                    pvacc = psV.tile([65, 2, 512], f32, tag="psv", name=f"pv{sc}_{g}")
                    pssE = psS.tile([128, 4, 256], f32, tag="pss", name=f"pssE{sc}_{g}")
                    pssO = psS.tile([128, 4, 256], f32, tag="pss", name=f"pssO{sc}_{g}")
                    for kt in range(4):
                        nc.tensor.matmul(pssE[:, kt, :],
                                         kc[0:64, g, kt * 128:(kt + 1) * 128],
                                         q2_T[0:64, g, b * 256:(b + 1) * 256],
                                         start=True, stop=True)
                        nc.tensor.matmul(pssO[:, kt, :],
                                         kc[64:128, g, kt * 128:(kt + 1) * 128],
                                         q2_T[64:128, g, b * 256:(b + 1) * 256],
                                         start=True, stop=True)
                    pTE = ptp.tile([128, 4, 256], bf16, tag="pT", name=f"pTE{sc}_{g}")
                    nc.scalar.activation(pTE[:], pssE[:], AF.Exp, scale=0.125)
                    pTO = ptp.tile([128, 4, 256], bf16, tag="pT", name=f"pTO{sc}_{g}")
                    nc.scalar.activation(pTO[:], pssO[:], AF.Exp, scale=0.125)
                    for kt in range(4):
                        nc.tensor.matmul(pvacc[:, 0, 0:256], vaug[:, kt, 2 * g, 0:65],
                                         pTE[:, kt, :], start=(kt == 0), stop=(kt == 3))
                    for kt in range(4):
                        nc.tensor.matmul(pvacc[:, 1, 0:256], vaug[:, kt, 2 * g + 1, 0:65],
                                         pTO[:, kt, :], start=(kt == 0), stop=(kt == 3))
                    if j == 0:
                        nc.vector.tensor_copy(acc[:, g * 2:(g + 1) * 2, :],
                                              pvacc[:, :, 0:256])
                    else:
                        nc.vector.tensor_tensor(acc[:, g * 2:(g + 1) * 2, :],
                                                acc[:, g * 2:(g + 1) * 2, :],
                                                pvacc[:, :, 0:256], ALU.add)

            # ================= emission =================
            # ---- P1: LN1(queries) -> qn_T (gain folded into wq/wk/wv) ----
            ln_to_dram(queries, qn_dram, FO, "l1")
            qn_T = fm.tile([128, FO, T], bf16, tag="fmT", name="qn_T")
            transpose_load(qn_T, qn_dram)

            k_chunk(0)
            k_chunk(1)

            # ---- P2: self-attn qkv (2 heads, feature-major) ----
            qs_T = selfp.tile([128, T], bf16, tag="qs_T")
            ks_T = selfp.tile([128, T], bf16, tag="ks_T")
            vs_T = selfp.tile([128, T], bf16, tag="vs_T")
            for wi, (w_sb, bias_sb, dst) in enumerate(((wq_sb, bq_sb, qs_T), (wk_sb, bk_sb, ks_T),
                                                      (wv_sb, None, vs_T))):
                for tc2 in range(2):
                    ps = psA.tile([128, 512], f32, tag="ps512", name=f"ps_qkv{wi}_{tc2}")
                    for f in range(FO):
                        nc.tensor.matmul(ps[:], w_sb[:, f, :], qn_T[:, f, tc2 * 512:(tc2 + 1) * 512],
                                         start=(f == 0), stop=(f == FO - 1))
                    if bias_sb is not None:
                        nc.scalar.activation(dst[:, tc2 * 512:(tc2 + 1) * 512], ps[:],
                                             AF.Identity, bias=bias_sb[:])
                    else:
                        nc.scalar.activation(dst[:, tc2 * 512:(tc2 + 1) * 512], ps[:], AF.Copy)
            vaug_s = selfp.tile([128, B, 2, HPC, 128], bf16, tag="vaug_s")
            nc.vector.memset(vaug_s[:, :, :, :, 64:65], 1.0)
            for b in range(B):
                for kt in range(2):
                    for hh in range(HPC):
                        nc.sync.dma_start_transpose(
                            vaug_s[:, b, kt, hh, 0:64],
                            vs_T[hh * 64:(hh + 1) * 64, b * 256 + kt * 128: b * 256 + (kt + 1) * 128])

            o_self = selfp.tile([128, T], bf16, tag="o_self")
            for b in range(B):
                for hh in range(HPC):
                    pso = psV.tile([65, 2, 512], f32, tag="psv", name=f"psoS_{b}_{hh}")
                    pss = psS.tile([128, 4, 256], f32, tag="pss", name=f"pssS_{b}_{hh}")
                    for kt in range(2):
                        nc.tensor.matmul(
                            pss[:, kt, :],
                            ks_T[hh * 64:(hh + 1) * 64, b * 256 + kt * 128: b * 256 + (kt + 1) * 128],
                            qs_T[hh * 64:(hh + 1) * 64, b * 256:(b + 1) * 256],
                            start=True, stop=True)
                    pT = sb.tile([128, 2, 256], bf16, tag="pT_s", name=f"pTS_{b}_{hh}")
                    nc.scalar.activation(pT[:], pss[:, 0:2, :], AF.Exp, scale=0.125)
                    for kt in range(2):
                        nc.tensor.matmul(pso[:, 0, 0:256], vaug_s[:, b, kt, hh, 0:65], pT[:, kt, :],
                                         start=(kt == 0), stop=(kt == 1))
                    rinv = sb.tile([1, 256], f32, tag="rinv", name=f"riS_{b}_{hh}")
                    nc.vector.reciprocal(rinv[:], pso[64:65, 0, 0:256])
                    rb = sb.tile([64, 256], f32, tag="rb", name=f"rbS_{b}_{hh}")
                    nc.gpsimd.partition_broadcast(rb[:], rinv[:])
                    oslice = o_self[hh * 64:(hh + 1) * 64, b * 256:(b + 1) * 256]
                    nc.vector.tensor_tensor(oslice, pso[0:64, 0, 0:256], rb[:], ALU.mult)
            if DBG:
                nc.sync.dma_start(dbg["dbg_oself"][:], o_self[:])
            ag1_in = dram.tile([128, T], bf16)
            ag1_out = dram.tile([NC, 128, T], bf16, addr_space="Shared")
            nc.sync.dma_start(ag1_in[:], o_self[:])
            nc.gpsimd.collective_compute(
                "AllGather", ALU.bypass, replica_groups=[list(range(NC))],
                ins=[ag1_in[:].opt()], outs=[ag1_out[:].opt()])

            k_chunk(2)

            o_full = fm.tile([128, FO, T], bf16, tag="fmT", name="o_full")
            for s in range(NC):
                nc.sync.dma_start(o_full[:, s, :], ag1_out[s])

            # ---- P3: sa_out projection (replicated) + residual -> x (DRAM) ----
            for oc in range(2):
                saow_c = wstream.tile([128, FO, 512], bf16, tag="wc", name=f"saow_c{oc}")
                nc.sync.dma_start(saow_c[:], saow_r[:, :, oc * 512:(oc + 1) * 512])
                for tt in range(FO):
                    ps = psA.tile([128, 512], f32, tag="ps512", name=f"ps_x{oc}_{tt}")
                    for s in range(FO):
                        nc.tensor.matmul(ps[:], o_full[:, s, tt * 128:(tt + 1) * 128],
                                         saow_c[:, s, :],
                                         start=(s == 0), stop=(s == FO - 1))
                    qres = sb.tile([128, 512], f32, tag="qres", name=f"qres{oc}_{tt}")
                    nc.sync.dma_start(qres[:], queries[tt * 128:(tt + 1) * 128, oc * 512:(oc + 1) * 512])
                    nc.vector.tensor_tensor(qres[:], ps[:], qres[:], ALU.add)
                    nc.vector.tensor_tensor(qres[:], qres[:], saob_bc[:, oc * 512:(oc + 1) * 512], ALU.add)
                    nc.sync.dma_start(dram_x[tt * 128:(tt + 1) * 128, oc * 512:(oc + 1) * 512], qres[:])

            if DBG:
                nc.sync.dma_start(dbg["dbg_x"][:], dram_x[:])
            # ---- P4: LN2(x) -> xn2_T (gain folded into cqw) ----
            ln_to_dram(dram_x, xn2_dram, FO, "l2")
            xn2_T = fm.tile([128, FO, T], bf16, tag="fmT", name="xn2_T")
            transpose_load(xn2_T, xn2_dram)

            # ---- P5: cross q projection (replicated, feature-major by head-pair) ----
            q2_T = q2p.tile([128, FO, T], bf16, tag="q2_T")
            for hf in range(2):
                cqw_c = wstream.tile([128, FO, 512], bf16, tag="wc", name=f"cqw_c{hf}")
                nc.sync.dma_start(cqw_c[:], cqw_r[:, :, hf * 512:(hf + 1) * 512])
                for p4 in range(4):
                    p = hf * 4 + p4
                    for th in range(2):
                        psq = psA.tile([128, 512], f32, tag="ps512", name=f"ps_q2{p}_{th}")
                        for f in range(FO):
                            nc.tensor.matmul(psq[:], cqw_c[:, f, p4 * 128:(p4 + 1) * 128],
                                             xn2_T[:, f, th * 512:(th + 1) * 512],
                                             start=(f == 0), stop=(f == FO - 1))
                        nc.scalar.activation(q2_T[:, p, th * 512:(th + 1) * 512], psq[:],
                                             AF.Identity, bias=cqb_sb[:, p:p + 1])
            if DBG:
                nc.sync.dma_start(dbg["dbg_q2"][:], q2_T[:])

            # ---- P6: cross-attention scores/PV, interleaved with remaining K/V ----
            for b in range(B):
                acc = accp.tile([65, H, 256], bf16, tag="acc", name=f"acc{b}")
                for j in range(2):
                    sc = b * 2 + j
                    v_chunk(sc)
                    spv_chunk(sc, acc)
                    if sc + 3 < NCH:
                        k_chunk(sc + 3)
                for half in range(2):
                    nc.sync.dma_start(dram_acc[2 * b + half],
                                      acc[:, :, half * 128:(half + 1) * 128])
                if DBG and b == B - 1:
                    nc.sync.dma_start(dbg["dbg_acc"][:], acc[:])

            # prefetch co weights (consumed right after the ReduceScatter)
            cow_cs = []
            for oc in range(2):
                cow_c = wstream.tile([128, FO, 512], bf16, tag="wc", name=f"cow_c{oc}")
                nc.sync.dma_start(cow_c[:], cow_r[:, :, oc * 512:(oc + 1) * 512])
                cow_cs.append(cow_c)

            # ---- P7: ReduceScatter + normalize own tokens ----
            rs_out = dram.tile([65, H, 128], bf16)
            nc.gpsimd.collective_compute(
                "ReduceScatter", ALU.add, replica_groups=[list(range(NC))],
                ins=[dram_acc[:].opt()], outs=[rs_out[:].opt()])
            rs_sb = tailp.tile([65, H, 128], bf16, tag="rs_sb")
            nc.sync.dma_start(rs_sb[:], rs_out[:])
            if DBG:
                nc.sync.dma_start(dbg["dbg_rs"][:], rs_sb[:])
            rinv = tailp.tile([1, H, 128], bf16, tag="rinv_c")
            with nc.allow_low_precision(reason="softmax denom reciprocal, 2e-2 budget"):
                nc.vector.reciprocal(rinv[:], rs_sb[64:65, :, :])
            rb = tailp.tile([64, H, 128], bf16, tag="rb_c")
            nc.gpsimd.partition_broadcast(rb[:], rinv[:])
            ocb = tailp.tile([128, FO, 128], bf16, tag="ocb")
            for h in range(H):
                nc.vector.tensor_tensor(ocb[(h % 2) * 64:(h % 2) * 64 + 64, h // 2, :],
                                        rs_sb[0:64, h, :], rb[:, h, :], ALU.mult)

            # ---- P8: co projection (own 128 tokens) + residual -> x3_own ----
            x_own = sb.tile([128, D], f32, tag="ln_in", name="x_own")
            nc.gpsimd.indirect_dma_start(
                out=x_own[:], out_offset=None, in_=dram_x[:, :],
                in_offset=IndirectOffsetOnAxis(ap=idx_sb[:], axis=0))
            x3_own = tailp.tile([128, D], f32, tag="x3_own")
            for oc in range(2):
                psy = psA.tile([128, 512], f32, tag="ps512", name=f"ps_x3{oc}")
                for f in range(FO):
                    nc.tensor.matmul(psy[:], ocb[:, f, :], cow_cs[oc][:, f, :],
                                     start=(f == 0), stop=(f == FO - 1))
                xsl = x3_own[:, oc * 512:(oc + 1) * 512]
                nc.vector.tensor_tensor(xsl, psy[:], x_own[:, oc * 512:(oc + 1) * 512], ALU.add)
                nc.vector.tensor_tensor(xsl, xsl, cob_bc[:, oc * 512:(oc + 1) * 512], ALU.add)
            if DBG:
                nc.sync.dma_start(dbg["dbg_x3"][:], x3_own[:])

            # ---- P9: LN3 (own tokens, SBUF-resident; gain folded into w1) ----
            xn3 = tailp.tile([128, D], bf16, tag="xn3")
            ln_tile(x3_own, xn3[:], "l3")
            xn3_T = tailp.tile([128, FO, 128], bf16, tag="xn3_T")
            for fo in range(FO):
                nc.sync.dma_start_transpose(xn3_T[:, fo, :], xn3[:, fo * 128:(fo + 1) * 128])

            # ---- P10: FFN on own tokens (layer 1 emitted hidden-major) ----
            h_T = tailp.tile([128, 32, 128], bf16, tag="h_T")
            for wc in range(8):
                w1c = wstream.tile([128, FO, 512], bf16, tag="wc", name=f"w1c{wc}")
                nc.sync.dma_start(w1c[:], w1_r[:, :, wc * 512:(wc + 1) * 512])
                psh = psA.tile([128, 4, 128], f32, tag="ps512", name=f"ps_h{wc}")
                for dt4 in range(4):
                    for f in range(FO):
                        nc.tensor.matmul(psh[:, dt4, :], w1c[:, f, dt4 * 128:(dt4 + 1) * 128],
                                         xn3_T[:, f, :],
                                         start=(f == 0), stop=(f == FO - 1))
                for dt4 in range(4):
                    dt = wc * 4 + dt4
                    nc.scalar.activation(h_T[:, dt, :], psh[:, dt4, :], AF.Gelu,
                                         bias=b1_sb[:, dt:dt + 1])
            out_sb = sb.tile([128, D], f32, tag="ln_in", name="out_sb")
            for oc in range(2):
                psy = psA.tile([128, 512], f32, tag="ps512", name=f"ps_y{oc}")
                for wc in range(4):
                    w2c = wstream.tile([128, FO, 512], bf16, tag="wc", name=f"w2c{oc}_{wc}")
                    nc.sync.dma_start(w2c[:], w2_r[:, wc * FO:(wc + 1) * FO, oc * 512:(oc + 1) * 512])
                    for jj in range(FO):
                        dt = wc * FO + jj
                        nc.tensor.matmul(psy[:], h_T[:, dt, :], w2c[:, jj, :],
                                         start=(dt == 0), stop=(dt == 31))
                ys = out_sb[:, oc * 512:(oc + 1) * 512]
                nc.vector.tensor_tensor(ys, psy[:], x3_own[:, oc * 512:(oc + 1) * 512], ALU.add)
                nc.vector.tensor_tensor(ys, ys, b2_bc[:, oc * 512:(oc + 1) * 512], ALU.add)
            nc.sync.dma_start(out[:], out_sb[:])

    nc.compile()
    return nc


def _pack_ln(v):
    return np.ascontiguousarray(np.asarray(v, dtype=np.float32).reshape(FO, 128).T)


def _get_nc():
    if "nc" not in _CACHE:
        _CACHE["nc"] = _build()
    return _CACHE["nc"]


def kernel(**inputs):
    nc = _get_nc()
    inp = {k: np.asarray(v) for k, v in inputs.items()}

    def bf(a):
        return np.ascontiguousarray(a).astype(BF16NP)

    def f8(a):
        return np.ascontiguousarray(a).astype(FP8NP)

    queries = np.ascontiguousarray(inp["queries"].reshape(T, D).astype(np.float32))
    context = inp["context"]  # [B, S, D]
    sa_in_w = inp["sa_in_w"]; sa_in_b = inp["sa_in_b"]
    ln1g, ln1b = inp["ln1_g"], inp["ln1_b"]
    ln2g, ln2b = inp["ln2_g"], inp["ln2_b"]
    ln3g, ln3b = inp["ln3_g"], inp["ln3_b"]

    # fold LN gains/biases into the consuming projections
    wq_f = sa_in_w[0 * D:1 * D] * ln1g[None, :]
    wk_f = sa_in_w[1 * D:2 * D] * ln1g[None, :]
    wv_f = sa_in_w[2 * D:3 * D] * ln1g[None, :]
    bq_f = sa_in_b[0 * D:1 * D] + sa_in_w[0 * D:1 * D] @ ln1b
    bk_f = sa_in_b[1 * D:2 * D] + sa_in_w[1 * D:2 * D] @ ln1b
    bv_f = sa_in_b[2 * D:3 * D] + sa_in_w[2 * D:3 * D] @ ln1b
    saob_eff = bf(inp["sa_out_b"] + inp["sa_out_w"] @ bv_f)
    cqw_f = inp["cq_w"] * ln2g[None, :]
    cqb_f = inp["cq_b"] + inp["cq_w"] @ ln2b
    cob_eff = bf(inp["co_b"] + inp["co_w"] @ inp["cv_b"])
    w1_f = inp["w1"] * ln3g[None, :]
    b1_f = inp["b1"] + inp["w1"] @ ln3b
    b1_p = np.ascontiguousarray(np.asarray(b1_f, np.float32).reshape(32, 128).T)

    shared = {
        "queries": queries,
        "saow": bf(inp["sa_out_w"].T), "saob": saob_eff,
        "cqw": bf(cqw_f.T), "ckw": f8(inp["ck_w"].T), "cvw": f8(inp["cv_w"].T),
        "cqb": _pack_ln(cqb_f), "ckb": _pack_ln(inp["ck_b"]),
        "cow": bf(inp["co_w"].T), "cob": cob_eff,
        "w1": bf(w1_f.T), "b1": b1_p,
        "w2": bf(inp["w2"].T), "b2": bf(inp["b2"]),
    }
    in_maps = []
    for c in range(NC):
        r = slice(c * 128, (c + 1) * 128)
        ctx_c = np.concatenate(
            [context[b, c * SPC:(c + 1) * SPC, :] for b in range(B)], axis=0)
        m = dict(shared)
        m.update({
            "ctx": f8(ctx_c.T),
            "wq": bf(wq_f[r].T), "bq": np.asarray(bq_f[r], np.float32),
            "wk": bf(wk_f[r].T), "bk": np.asarray(bk_f[r], np.float32),
            "wv": bf(wv_f[r].T),
            "own_idx": (c * 128 + np.arange(128, dtype=np.int32)).reshape(128, 1),
        })
        in_maps.append(m)

    res = run_bass_kernel_spmd(nc, in_maps, core_ids=list(range(NC)),
                               **_CACHE.get("run_kwargs", {}))
    _CACHE["last_result"] = res
    out = np.concatenate([np.asarray(res.results[c]["out"]) for c in range(NC)], axis=0)
    return out.reshape(B, Q, D).astype(np.float32)
